# revision 5
# baseline (speedup 1.0000x reference)
"""Trainium2 Bass kernel for the retrieval-KNN correlation problem.

Problem (per batch element b):
    idx[k,p]   = x[b,k,p] + 64*y[b,k,p]              (pixel coords into ref map)
    S[k,p]     = sum_c ref[b,c,idx[k,p]] * inp[b,c,p]
    best[p]    = argmax_k S[k,p]        (first occurrence on ties)
    out_x[p]   = x[b,best[p],p],  out_y[p] = y[b,best[p],p]

Sharding: 8 cores = (batch b = core//2, pixel half = core%2). Each core owns
all 16 candidates for 2048 contiguous pixels of one batch element, so there is
no cross-core communication.

Per-core dataflow:
  - ref[b] (256,4096) resident in SBUF as two (128,4096) channel tiles.
  - GPSIMD ap_gather pulls ref columns at idx[k,:] (indices shared across
    partitions, wrapped per 16-partition group).
  - DVE multiplies gathered columns with inp tiles.
  - PE reduces over channels via one-hot-column matmuls (float32r, full rate)
    accumulating S (16,2048) in PSUM.
  - PE transposes S to pixel-major (128,16) tiles; DVE computes the
    first-occurrence argmax via a reverse-weight trick and selects x/y.
"""

import numpy as np
from contextlib import ExitStack

import concourse.bacc as bacc
import concourse.bass as bass
import concourse.mybir as mybir
import concourse.tile as tile
from concourse import bass_utils

B, K, CN, H, W = 4, 16, 256, 64, 64
HW = H * W            # 4096 pixels per batch element
HALF = HW // 2        # 2048 pixels per core
NCORES = 8
NT = HALF // 128      # 16 pixel tiles of 128

f32 = mybir.dt.float32
f32r = mybir.dt.float32r
i16 = mybir.dt.int16


def build_program():
    nc = bacc.Bacc("TRN2", target_bir_lowering=False, debug=False)

    ref_d = nc.dram_tensor("ref", (CN, HW), f32, kind="ExternalInput")
    inp_d = nc.dram_tensor("inp", (CN, HALF), f32, kind="ExternalInput")
    xh_d = nc.dram_tensor("xh", (K, HALF), f32, kind="ExternalInput")
    yh_d = nc.dram_tensor("yh", (K, HALF), f32, kind="ExternalInput")
    ident_d = nc.dram_tensor("ident", (128, 128), f32, kind="ExternalInput")
    eye16_d = nc.dram_tensor("eye16", (128, 16 * K), f32, kind="ExternalInput")
    revc_d = nc.dram_tensor("revc", (128, 16 * NT), f32, kind="ExternalInput")
    ox_d = nc.dram_tensor("ox", (128, NT), f32, kind="ExternalOutput")
    oy_d = nc.dram_tensor("oy", (128, NT), f32, kind="ExternalOutput")

    with ExitStack() as ctx:
        tc = ctx.enter_context(tile.TileContext(nc))
        pers = ctx.enter_context(tc.tile_pool(name="pers", bufs=1))
        gpool = ctx.enter_context(tc.tile_pool(name="g", bufs=3))
        tpool = ctx.enter_context(tc.tile_pool(name="t", bufs=3))
        ps_s = ctx.enter_context(tc.tile_pool(name="ps_s", bufs=1, space="PSUM"))
        ps_tp = ctx.enter_context(tc.tile_pool(name="ps_tp", bufs=2, space="PSUM"))

        # ---- persistent tiles -------------------------------------------------
        rt = [pers.tile([128, HW], f32, tag=f"ref{c}", name=f"ref{c}") for c in range(2)]
        it = [pers.tile([128, HALF], f32, tag=f"inp{c}", name=f"inp{c}") for c in range(2)]
        xn = pers.tile([K, HALF], f32, tag="xn")
        yn = pers.tile([K, HALF], f32, tag="yn")
        rx = pers.tile([128, 16 * K], f32, tag="rx")
        ry = pers.tile([128, 16 * K], f32, tag="ry")
        ri = pers.tile([128, 16 * K], f32, tag="ri")     # idx in R-layout, fp32
        ident = pers.tile([128, 128], f32, tag="ident")
        eye16 = pers.tile([128, 16 * K], f32, tag="eye16")
        revc = pers.tile([128, 16 * NT], f32, tag="revc")
        wk = [pers.tile([128, 128], i16, tag=f"w{k}", name=f"w{k}") for k in range(K)]
        xt = pers.tile([128, 16 * NT], f32, tag="xt")    # x, pixel-major
        yt = pers.tile([128, 16 * NT], f32, tag="yt")
        st = pers.tile([128, 16 * NT], f32, tag="st")    # S^T, pixel-major
        ssb = pers.tile([K, HALF], f32, tag="ssb")

        # ---- loads ------------------------------------------------------------
        nc.sync.dma_start(ident[:], ident_d.ap())
        nc.sync.dma_start(eye16[:], eye16_d.ap())
        nc.sync.dma_start(revc[:], revc_d.ap())
        for c in range(2):
            nc.sync.dma_start(rt[c][:], ref_d.ap()[c * 128:(c + 1) * 128, :])
            nc.sync.dma_start(it[c][:], inp_d.ap()[c * 128:(c + 1) * 128, :])
        nc.sync.dma_start(xn[:], xh_d.ap())
        nc.sync.dma_start(yn[:], yh_d.ap())
        # R-layout: rx[r, 16k+c] = x[k, 16r+c] (contiguous 64B runs)
        nc.sync.dma_start(rx[:].rearrange("p (k c) -> p k c", c=16),
                          xh_d.ap().rearrange("k (r c) -> r k c", c=16))
        nc.sync.dma_start(ry[:].rearrange("p (k c) -> p k c", c=16),
                          yh_d.ap().rearrange("k (r c) -> r k c", c=16))

        ident16 = ident[0:16, 0:16]

        # ---- index pipeline: idx = x + 64*y, wrapped per-k index tiles --------
        nc.vector.tensor_scalar_mul(ri[:], ry[:], 64.0)
        nc.vector.tensor_add(ri[:], ri[:], rx[:])
        for k in range(K):
            rep = tpool.tile([128, 128], f32, tag="rep")
            src = ri[:, 16 * k:16 * (k + 1)].unsqueeze(1).broadcast_to((128, 8, 16))
            nc.vector.tensor_copy(rep[:].rearrange("p (g c) -> p g c", c=16), src)
            wp = ps_tp.tile([128, 128], f32, tag="tp")
            nc.tensor.transpose(wp[:], rep[:], ident[:])
            nc.scalar.copy(wk[k][:], wp[:])

        # ---- x/y to pixel-major (128,16) tiles via PE transpose ---------------
        for t in range(NT):
            xp = ps_tp.tile([128, 16], f32, tag="tp")
            nc.tensor.transpose(xp[:], xn[:, 128 * t:128 * (t + 1)], ident16)
            nc.scalar.copy(xt[:, 16 * t:16 * (t + 1)], xp[:])
            yp = ps_tp.tile([128, 16], f32, tag="tp")
            nc.tensor.transpose(yp[:], yn[:, 128 * t:128 * (t + 1)], ident16)
            nc.scalar.copy(yt[:, 16 * t:16 * (t + 1)], yp[:])

        # ---- main loop: gather, multiply, reduce over channels ----------------
        s_ps = ps_s.tile([K, HALF], f32, tag="s")
        for k in range(K):
            for c in range(2):
                g = gpool.tile([128, HALF], f32, tag="g")
                nc.gpsimd.ap_gather(
                    g[:], rt[c][:], wk[k][:],
                    channels=128, num_elems=HW, d=1, num_idxs=HALF,
                )
                tt = tpool.tile([128, HALF], f32, tag="t")
                nc.vector.tensor_mul(tt[:], g[:], it[c][:])
                for q in range(HALF // 512):
                    nc.tensor.matmul(
                        s_ps[:, 512 * q:512 * (q + 1)],
                        lhsT=eye16[:, 16 * k:16 * (k + 1)],
                        rhs=tt[:, 512 * q:512 * (q + 1)],
                        start=(k == 0 and c == 0),
                        stop=(k == K - 1 and c == 1),
                    )

        # ---- S -> pixel-major S^T tiles ---------------------------------------
        nc.scalar.copy(ssb[:], s_ps[:])
        for t in range(NT):
            stp = ps_tp.tile([128, 16], f32, tag="tp")
            nc.tensor.transpose(stp[:], ssb[:, 128 * t:128 * (t + 1)], ident16)
            nc.scalar.copy(st[:, 16 * t:16 * (t + 1)], stp[:])

        # ---- argmax (first occurrence) + offset select ------------------------
        def grp(ap):  # (128, 256) -> (128, 16, 16)
            return ap.rearrange("p (t j) -> p t j", j=16)

        gmax = pers.tile([128, NT], f32, tag="gmax")
        ohall = pers.tile([128, 16 * NT], f32, tag="ohall")
        t1 = pers.tile([128, 16 * NT], f32, tag="t1")
        r1 = pers.tile([128, NT], f32, tag="r1")
        oh1 = pers.tile([128, 16 * NT], f32, tag="oh1")
        sel = pers.tile([128, 16 * NT], f32, tag="sel")
        oxv = pers.tile([128, NT], f32, tag="oxv")
        oyv = pers.tile([128, NT], f32, tag="oyv")

        nc.vector.tensor_reduce(gmax[:], grp(st[:]), axis=mybir.AxisListType.X,
                                op=mybir.AluOpType.max)
        gb = gmax[:].unsqueeze(2).broadcast_to((128, NT, 16))
        nc.vector.tensor_tensor(grp(ohall[:]), grp(st[:]), gb,
                                op=mybir.AluOpType.is_equal)
        nc.vector.tensor_mul(t1[:], ohall[:], revc[:])
        nc.vector.tensor_reduce(r1[:], grp(t1[:]), axis=mybir.AxisListType.X,
                                op=mybir.AluOpType.max)
        rb = r1[:].unsqueeze(2).broadcast_to((128, NT, 16))
        nc.vector.tensor_tensor(grp(oh1[:]), grp(t1[:]), rb,
                                op=mybir.AluOpType.is_equal)
        nc.vector.tensor_mul(sel[:], oh1[:], xt[:])
        nc.vector.tensor_reduce(oxv[:], grp(sel[:]), axis=mybir.AxisListType.X,
                                op=mybir.AluOpType.add)
        nc.vector.tensor_mul(sel[:], oh1[:], yt[:])
        nc.vector.tensor_reduce(oyv[:], grp(sel[:]), axis=mybir.AxisListType.X,
                                op=mybir.AluOpType.add)

        nc.sync.dma_start(ox_d.ap(), oxv[:])
        nc.sync.dma_start(oy_d.ap(), oyv[:])

    nc.compile()
    return nc


def make_consts():
    ident = np.eye(128, dtype=np.float32)
    eye16 = np.tile(np.eye(16, dtype=np.float32).reshape(1, 256), (128, 1))
    revc = np.tile(
        np.tile((16.0 - np.arange(16, dtype=np.float32)), NT).reshape(1, 16 * NT),
        (128, 1),
    )
    return ident, eye16, np.ascontiguousarray(revc)


def make_in_maps(input_features, ref_features, aggregated_x, aggregated_y):
    ident, eye16, revc = make_consts()
    in_maps = []
    for core in range(NCORES):
        b, h = core // 2, core % 2
        sl = slice(h * HALF, (h + 1) * HALF)
        in_maps.append({
            "ref": np.ascontiguousarray(ref_features[b].reshape(CN, HW)),
            "inp": np.ascontiguousarray(input_features[b].reshape(CN, HW)[:, sl]),
            "xh": np.ascontiguousarray(aggregated_x[b].reshape(K, HW)[:, sl]),
            "yh": np.ascontiguousarray(aggregated_y[b].reshape(K, HW)[:, sl]),
            "ident": ident,
            "eye16": eye16,
            "revc": revc,
        })
    return in_maps


def assemble_outputs(results):
    offset_x = np.empty((B, 1, H, W), dtype=np.float32)
    offset_y = np.empty((B, 1, H, W), dtype=np.float32)
    for core in range(NCORES):
        b, h = core // 2, core % 2
        sl = slice(h * HALF, (h + 1) * HALF)
        # ox[p, t] holds pixel t*128+p -> transpose to pixel order
        offset_x[b, 0].reshape(HW)[sl] = results[core]["ox"].T.reshape(HALF)
        offset_y[b, 0].reshape(HW)[sl] = results[core]["oy"].T.reshape(HALF)
    return offset_x, offset_y


_PROGRAM = None


def kernel(input_features, ref_features, aggregated_x, aggregated_y):
    global _PROGRAM
    if _PROGRAM is None:
        _PROGRAM = build_program()
    nc = _PROGRAM
    in_maps = make_in_maps(input_features, ref_features, aggregated_x, aggregated_y)
    res = bass_utils.run_bass_kernel_spmd(nc, in_maps, core_ids=list(range(NCORES)))
    return assemble_outputs(res.results)


# revision 9
# speedup vs baseline: 1.0001x; 1.0001x over previous
"""Trainium2 Bass kernel for the retrieval-KNN correlation problem.

Problem (per batch element b):
    idx[k,p]   = x[b,k,p] + 64*y[b,k,p]              (pixel coords into ref map)
    S[k,p]     = sum_c ref[b,c,idx[k,p]] * inp[b,c,p]
    best[p]    = argmax_k S[k,p]        (first occurrence on ties)
    out_x[p]   = x[b,best[p],p],  out_y[p] = y[b,best[p],p]

Sharding: 8 cores = (batch b = core//2, pixel half = core%2). Each core owns
all 16 candidates for 2048 contiguous pixels of one batch element, so there is
no cross-core communication.

Per-core dataflow:
  - ref[b] (256,4096) resident in SBUF as two (128,4096) channel tiles.
  - GPSIMD ap_gather pulls ref columns at idx[k,:] (indices shared across
    partitions, wrapped per 16-partition group).
  - DVE multiplies gathered columns with inp tiles.
  - PE reduces over channels via one-hot-column matmuls (float32r, full rate)
    accumulating S (16,2048) in PSUM.
  - PE transposes S to pixel-major (128,16) tiles; DVE computes the
    first-occurrence argmax via a reverse-weight trick and selects x/y.
"""

import numpy as np
from contextlib import ExitStack

import concourse.bacc as bacc
import concourse.bass as bass
import concourse.mybir as mybir
import concourse.tile as tile
from concourse import bass_utils

B, K, CN, H, W = 4, 16, 256, 64, 64
HW = H * W            # 4096 pixels per batch element
HALF = HW // 2        # 2048 pixels per core
NCORES = 8
NT = HALF // 128      # 16 pixel tiles of 128

f32 = mybir.dt.float32
f32r = mybir.dt.float32r
i16 = mybir.dt.int16


def build_program():
    nc = bacc.Bacc("TRN2", target_bir_lowering=False, debug=False)

    ref_d = nc.dram_tensor("ref", (CN, HW), f32, kind="ExternalInput")
    inp_d = nc.dram_tensor("inp", (CN, HALF), f32, kind="ExternalInput")
    xh_d = nc.dram_tensor("xh", (K, HALF), f32, kind="ExternalInput")
    yh_d = nc.dram_tensor("yh", (K, HALF), f32, kind="ExternalInput")
    ident_d = nc.dram_tensor("ident", (128, 128), f32, kind="ExternalInput")
    eye16_d = nc.dram_tensor("eye16", (128, 16 * K), f32, kind="ExternalInput")
    revc_d = nc.dram_tensor("revc", (128, 16 * NT), f32, kind="ExternalInput")
    ox_d = nc.dram_tensor("ox", (128, NT), f32, kind="ExternalOutput")
    oy_d = nc.dram_tensor("oy", (128, NT), f32, kind="ExternalOutput")

    with ExitStack() as ctx:
        tc = ctx.enter_context(tile.TileContext(nc))
        pers = ctx.enter_context(tc.tile_pool(name="pers", bufs=1))
        gpool = ctx.enter_context(tc.tile_pool(name="g", bufs=3))
        tpool = ctx.enter_context(tc.tile_pool(name="t", bufs=3))
        ps_s = ctx.enter_context(tc.tile_pool(name="ps_s", bufs=1, space="PSUM"))
        ps_tp = ctx.enter_context(tc.tile_pool(name="ps_tp", bufs=2, space="PSUM"))

        # ---- persistent tiles -------------------------------------------------
        rt = [pers.tile([128, HW], f32, tag=f"ref{c}", name=f"ref{c}") for c in range(2)]
        it = [pers.tile([128, HALF], f32, tag=f"inp{c}", name=f"inp{c}") for c in range(2)]
        xn = pers.tile([K, HALF], f32, tag="xn")
        yn = pers.tile([K, HALF], f32, tag="yn")
        ri = pers.tile([128, 16 * K], f32, tag="ri")     # idx in R-layout, fp32
        ident = pers.tile([128, 128], f32, tag="ident")
        eye16 = pers.tile([128, 16 * K], f32, tag="eye16")
        revc = pers.tile([128, 16 * NT], f32, tag="revc")
        wk = [pers.tile([128, 128], i16, tag=f"w{k}", name=f"w{k}") for k in range(K)]
        xt = pers.tile([128, 16 * NT], f32, tag="xt")    # x, pixel-major
        yt = pers.tile([128, 16 * NT], f32, tag="yt")
        st = pers.tile([128, 16 * NT], f32, tag="st")    # S^T, pixel-major
        ssb = pers.tile([K, HALF], f32, tag="ssb")

        # ---- loads ------------------------------------------------------------
        nc.sync.dma_start(ident[:], ident_d.ap())
        nc.sync.dma_start(eye16[:], eye16_d.ap())
        nc.sync.dma_start(revc[:], revc_d.ap())
        for c in range(2):
            nc.sync.dma_start(rt[c][:], ref_d.ap()[c * 128:(c + 1) * 128, :])
            nc.sync.dma_start(it[c][:], inp_d.ap()[c * 128:(c + 1) * 128, :])
        nc.sync.dma_start(xn[:], xh_d.ap())
        nc.sync.dma_start(yn[:], yh_d.ap())

        ident16 = ident[0:16, 0:16]

        # ---- index pipeline: idx = x + 64*y in R-layout -----------------------
        # ri[r, 16k+c] = idx[k, 16r+c], built via strided PE transposes of the
        # contiguous xn/yn loads (a strided DMA here would cost ~4096 64B
        # descriptors at ~600ns each).
        xn_g = xn[:].rearrange("k (r c) -> k r c", c=16)
        yn_g = yn[:].rearrange("k (r c) -> k r c", c=16)
        ri_g = ri[:].rearrange("p (k c) -> p k c", c=16)
        for c16 in range(16):
            txp = ps_tp.tile([128, K], f32, tag="tp", name=f"txp{c16}")
            nc.tensor.transpose(txp[:], xn_g[:, :, c16], ident16)
            typ = ps_tp.tile([128, K], f32, tag="tp2", name=f"typ{c16}")
            nc.tensor.transpose(typ[:], yn_g[:, :, c16], ident16)
            rtmp = tpool.tile([128, K], f32, tag="rtmp", name=f"rtmp{c16}")
            nc.vector.tensor_scalar_mul(rtmp[:], typ[:], 64.0)
            nc.vector.tensor_add(ri_g[:, :, c16], rtmp[:], txp[:])
        for k in range(K):
            rep = tpool.tile([128, 128], f32, tag="rep")
            src = ri[:, 16 * k:16 * (k + 1)].unsqueeze(1).broadcast_to((128, 8, 16))
            nc.vector.tensor_copy(rep[:].rearrange("p (g c) -> p g c", c=16), src)
            wp = ps_tp.tile([128, 128], f32, tag="tp")
            nc.tensor.transpose(wp[:], rep[:], ident[:])
            nc.scalar.copy(wk[k][:], wp[:])

        # ---- x/y to pixel-major (128,16) tiles via PE transpose ---------------
        for t in range(NT):
            xp = ps_tp.tile([128, 16], f32, tag="tp")
            nc.tensor.transpose(xp[:], xn[:, 128 * t:128 * (t + 1)], ident16)
            nc.scalar.copy(xt[:, 16 * t:16 * (t + 1)], xp[:])
            yp = ps_tp.tile([128, 16], f32, tag="tp")
            nc.tensor.transpose(yp[:], yn[:, 128 * t:128 * (t + 1)], ident16)
            nc.scalar.copy(yt[:, 16 * t:16 * (t + 1)], yp[:])

        # ---- main loop: gather, multiply, reduce over channels ----------------
        s_ps = ps_s.tile([K, HALF], f32, tag="s")
        for k in range(K):
            for c in range(2):
                g = gpool.tile([128, HALF], f32, tag="g")
                nc.gpsimd.ap_gather(
                    g[:], rt[c][:], wk[k][:],
                    channels=128, num_elems=HW, d=1, num_idxs=HALF,
                )
                tt = tpool.tile([128, HALF], f32, tag="t")
                nc.vector.tensor_mul(tt[:], g[:], it[c][:])
                for q in range(HALF // 512):
                    nc.tensor.matmul(
                        s_ps[:, 512 * q:512 * (q + 1)],
                        lhsT=eye16[:, 16 * k:16 * (k + 1)],
                        rhs=tt[:, 512 * q:512 * (q + 1)],
                        start=(k == 0 and c == 0),
                        stop=(k == K - 1 and c == 1),
                    )

        # ---- S -> pixel-major S^T tiles ---------------------------------------
        nc.scalar.copy(ssb[:], s_ps[:])
        for t in range(NT):
            stp = ps_tp.tile([128, 16], f32, tag="tp")
            nc.tensor.transpose(stp[:], ssb[:, 128 * t:128 * (t + 1)], ident16)
            nc.scalar.copy(st[:, 16 * t:16 * (t + 1)], stp[:])

        # ---- argmax (first occurrence) + offset select ------------------------
        def grp(ap):  # (128, 256) -> (128, 16, 16)
            return ap.rearrange("p (t j) -> p t j", j=16)

        gmax = pers.tile([128, NT], f32, tag="gmax")
        ohall = pers.tile([128, 16 * NT], f32, tag="ohall")
        t1 = pers.tile([128, 16 * NT], f32, tag="t1")
        r1 = pers.tile([128, NT], f32, tag="r1")
        oh1 = pers.tile([128, 16 * NT], f32, tag="oh1")
        sel = pers.tile([128, 16 * NT], f32, tag="sel")
        oxv = pers.tile([128, NT], f32, tag="oxv")
        oyv = pers.tile([128, NT], f32, tag="oyv")

        nc.vector.tensor_reduce(gmax[:], grp(st[:]), axis=mybir.AxisListType.X,
                                op=mybir.AluOpType.max)
        gb = gmax[:].unsqueeze(2).broadcast_to((128, NT, 16))
        nc.vector.tensor_tensor(grp(ohall[:]), grp(st[:]), gb,
                                op=mybir.AluOpType.is_equal)
        nc.vector.tensor_mul(t1[:], ohall[:], revc[:])
        nc.vector.tensor_reduce(r1[:], grp(t1[:]), axis=mybir.AxisListType.X,
                                op=mybir.AluOpType.max)
        rb = r1[:].unsqueeze(2).broadcast_to((128, NT, 16))
        nc.vector.tensor_tensor(grp(oh1[:]), grp(t1[:]), rb,
                                op=mybir.AluOpType.is_equal)
        nc.vector.tensor_mul(sel[:], oh1[:], xt[:])
        nc.vector.tensor_reduce(oxv[:], grp(sel[:]), axis=mybir.AxisListType.X,
                                op=mybir.AluOpType.add)
        nc.vector.tensor_mul(sel[:], oh1[:], yt[:])
        nc.vector.tensor_reduce(oyv[:], grp(sel[:]), axis=mybir.AxisListType.X,
                                op=mybir.AluOpType.add)

        nc.sync.dma_start(ox_d.ap(), oxv[:])
        nc.sync.dma_start(oy_d.ap(), oyv[:])

    nc.compile()
    return nc


def make_consts():
    ident = np.eye(128, dtype=np.float32)
    eye16 = np.tile(np.eye(16, dtype=np.float32).reshape(1, 256), (128, 1))
    revc = np.tile(
        np.tile((16.0 - np.arange(16, dtype=np.float32)), NT).reshape(1, 16 * NT),
        (128, 1),
    )
    return ident, eye16, np.ascontiguousarray(revc)


def make_in_maps(input_features, ref_features, aggregated_x, aggregated_y):
    ident, eye16, revc = make_consts()
    in_maps = []
    for core in range(NCORES):
        b, h = core // 2, core % 2
        sl = slice(h * HALF, (h + 1) * HALF)
        in_maps.append({
            "ref": np.ascontiguousarray(ref_features[b].reshape(CN, HW)),
            "inp": np.ascontiguousarray(input_features[b].reshape(CN, HW)[:, sl]),
            "xh": np.ascontiguousarray(aggregated_x[b].reshape(K, HW)[:, sl]),
            "yh": np.ascontiguousarray(aggregated_y[b].reshape(K, HW)[:, sl]),
            "ident": ident,
            "eye16": eye16,
            "revc": revc,
        })
    return in_maps


def assemble_outputs(results):
    offset_x = np.empty((B, 1, H, W), dtype=np.float32)
    offset_y = np.empty((B, 1, H, W), dtype=np.float32)
    for core in range(NCORES):
        b, h = core // 2, core % 2
        sl = slice(h * HALF, (h + 1) * HALF)
        # ox[p, t] holds pixel t*128+p -> transpose to pixel order
        offset_x[b, 0].reshape(HW)[sl] = results[core]["ox"].T.reshape(HALF)
        offset_y[b, 0].reshape(HW)[sl] = results[core]["oy"].T.reshape(HALF)
    return offset_x, offset_y


_PROGRAM = None


def kernel(input_features, ref_features, aggregated_x, aggregated_y):
    global _PROGRAM
    if _PROGRAM is None:
        _PROGRAM = build_program()
    nc = _PROGRAM
    in_maps = make_in_maps(input_features, ref_features, aggregated_x, aggregated_y)
    res = bass_utils.run_bass_kernel_spmd(nc, in_maps, core_ids=list(range(NCORES)))
    return assemble_outputs(res.results)


# revision 12
# speedup vs baseline: 1.8692x; 1.8691x over previous
"""Trainium2 Bass kernel for the retrieval-KNN correlation problem.

Problem (per batch element b):
    idx[k,p]   = x[b,k,p] + 64*y[b,k,p]              (pixel coords into ref map)
    S[k,p]     = sum_c ref[b,c,idx[k,p]] * inp[b,c,p]
    best[p]    = argmax_k S[k,p]        (first occurrence on ties)
    out_x[p]   = x[b,best[p],p],  out_y[p] = y[b,best[p],p]

Sharding: 8 cores = (batch b = core//2, pixel half = core%2). Each core owns
all 16 candidates for 2048 contiguous pixels of one batch element, so there is
no cross-core communication.

Per-core dataflow:
  - ref[b] (256,4096) resident in SBUF as two (128,4096) channel tiles.
  - GPSIMD ap_gather pulls ref columns at idx[k,:] (indices shared across
    partitions, wrapped per 16-partition group).
  - DVE multiplies gathered columns with inp tiles.
  - PE reduces over channels via one-hot-column matmuls (float32r, full rate)
    accumulating S (16,2048) in PSUM.
  - PE transposes S to pixel-major (128,16) tiles; DVE computes the
    first-occurrence argmax via a reverse-weight trick and selects x/y.
"""

import numpy as np
from contextlib import ExitStack

import concourse.bacc as bacc
import concourse.bass as bass
import concourse.mybir as mybir
import concourse.tile as tile
from concourse import bass_utils

B, K, CN, H, W = 4, 16, 256, 64, 64
HW = H * W            # 4096 pixels per batch element
HALF = HW // 2        # 2048 pixels per core
NCORES = 8
NT = HALF // 128      # 16 pixel tiles of 128

f32 = mybir.dt.float32
f32r = mybir.dt.float32r
i16 = mybir.dt.int16


def build_program():
    nc = bacc.Bacc("TRN2", target_bir_lowering=False, debug=False)

    ref_d = nc.dram_tensor("ref", (128, 2 * HW), f32, kind="ExternalInput")
    inp_d = nc.dram_tensor("inp", (128, 2 * HALF), f32, kind="ExternalInput")
    xh_d = nc.dram_tensor("xh", (K, HALF), f32, kind="ExternalInput")
    yh_d = nc.dram_tensor("yh", (K, HALF), f32, kind="ExternalInput")
    ident_d = nc.dram_tensor("ident", (128, 128), f32, kind="ExternalInput")
    eye16_d = nc.dram_tensor("eye16", (128, 16 * K), f32, kind="ExternalInput")
    revc_d = nc.dram_tensor("revc", (128, 16 * NT), f32, kind="ExternalInput")
    ox_d = nc.dram_tensor("ox", (128, NT), f32, kind="ExternalOutput")
    oy_d = nc.dram_tensor("oy", (128, NT), f32, kind="ExternalOutput")

    with ExitStack() as ctx:
        tc = ctx.enter_context(tile.TileContext(nc))
        pers = ctx.enter_context(tc.tile_pool(name="pers", bufs=1))
        gpool = ctx.enter_context(tc.tile_pool(name="g", bufs=3))
        tpool = ctx.enter_context(tc.tile_pool(name="t", bufs=3))
        ps_s = ctx.enter_context(tc.tile_pool(name="ps_s", bufs=1, space="PSUM"))
        ps_tp = ctx.enter_context(tc.tile_pool(name="ps_tp", bufs=2, space="PSUM"))

        # ---- persistent tiles -------------------------------------------------
        rt = pers.tile([128, 2 * HW], f32, tag="refp")
        it = pers.tile([128, 2 * HALF], f32, tag="inpp")
        xn = pers.tile([K, HALF], f32, tag="xn")
        yn = pers.tile([K, HALF], f32, tag="yn")
        ri = pers.tile([128, 16 * K], f32, tag="ri")     # idx in R-layout, fp32
        ident = pers.tile([128, 128], f32, tag="ident")
        eye16 = pers.tile([128, 16 * K], f32, tag="eye16")
        revc = pers.tile([128, 16 * NT], f32, tag="revc")
        wk = [pers.tile([128, 128], i16, tag=f"w{k}", name=f"w{k}") for k in range(K)]
        xt = pers.tile([128, 16 * NT], f32, tag="xt")    # x, pixel-major
        yt = pers.tile([128, 16 * NT], f32, tag="yt")
        st = pers.tile([128, 16 * NT], f32, tag="st")    # S^T, pixel-major
        ssb = pers.tile([K, HALF], f32, tag="ssb")

        # ---- loads ------------------------------------------------------------
        nc.sync.dma_start(ident[:], ident_d.ap())
        nc.sync.dma_start(eye16[:], eye16_d.ap())
        nc.sync.dma_start(revc[:], revc_d.ap())
        nc.sync.dma_start(rt[:], ref_d.ap())
        nc.sync.dma_start(it[:], inp_d.ap())
        nc.sync.dma_start(xn[:], xh_d.ap())
        nc.sync.dma_start(yn[:], yh_d.ap())

        ident16 = ident[0:16, 0:16]

        # ---- index pipeline: idx = x + 64*y in R-layout -----------------------
        # ri[r, 16k+c] = idx[k, 16r+c], built via strided PE transposes of the
        # contiguous xn/yn loads (a strided DMA here would cost ~4096 64B
        # descriptors at ~600ns each).
        xn_g = xn[:].rearrange("k (r c) -> k r c", c=16)
        yn_g = yn[:].rearrange("k (r c) -> k r c", c=16)
        ri_g = ri[:].rearrange("p (k c) -> p k c", c=16)
        for c16 in range(16):
            txp = ps_tp.tile([128, K], f32, tag="tp", name=f"txp{c16}")
            nc.tensor.transpose(txp[:], xn_g[:, :, c16], ident16)
            typ = ps_tp.tile([128, K], f32, tag="tp2", name=f"typ{c16}")
            nc.tensor.transpose(typ[:], yn_g[:, :, c16], ident16)
            rtmp = tpool.tile([128, K], f32, tag="rtmp", name=f"rtmp{c16}")
            nc.vector.tensor_scalar_mul(rtmp[:], typ[:], 64.0)
            nc.vector.tensor_add(ri_g[:, :, c16], rtmp[:], txp[:])
        for k in range(K):
            rep = tpool.tile([128, 128], f32, tag="rep")
            src = ri[:, 16 * k:16 * (k + 1)].unsqueeze(1).broadcast_to((128, 8, 16))
            nc.vector.tensor_copy(rep[:].rearrange("p (g c) -> p g c", c=16), src)
            wp = ps_tp.tile([128, 128], f32, tag="tp")
            nc.tensor.transpose(wp[:], rep[:], ident[:])
            nc.scalar.copy(wk[k][:], wp[:])

        # ---- x/y to pixel-major (128,16) tiles via PE transpose ---------------
        for t in range(NT):
            xp = ps_tp.tile([128, 16], f32, tag="tp")
            nc.tensor.transpose(xp[:], xn[:, 128 * t:128 * (t + 1)], ident16)
            nc.scalar.copy(xt[:, 16 * t:16 * (t + 1)], xp[:])
            yp = ps_tp.tile([128, 16], f32, tag="tp")
            nc.tensor.transpose(yp[:], yn[:, 128 * t:128 * (t + 1)], ident16)
            nc.scalar.copy(yt[:, 16 * t:16 * (t + 1)], yp[:])

        # ---- main loop: gather, multiply, reduce over channels ----------------
        s_ps = ps_s.tile([K, HALF], f32, tag="s")
        for k in range(K):
            g = gpool.tile([128, 2 * HALF], f32, tag="g")
            nc.gpsimd.ap_gather(
                g[:].rearrange("p (i d) -> p i d", d=2),
                rt[:].rearrange("p (e d) -> p e d", d=2),
                wk[k][:],
                channels=128, num_elems=HW, d=2, num_idxs=HALF,
            )
            # de-interleave at the multiply (strided reads, contiguous writes)
            # so the pair-sum accumulates via two matmul passes into one
            # (16, HALF) PSUM tile
            tt = tpool.tile([128, 2 * HALF], f32, tag="t")
            gv = g[:].rearrange("p (i d) -> p i d", d=2)
            iv = it[:].rearrange("p (i d) -> p i d", d=2)
            for par in range(2):
                nc.vector.tensor_mul(tt[:, par * HALF:(par + 1) * HALF],
                                     gv[:, :, par], iv[:, :, par])
            for par in range(2):
                for q in range(HALF // 512):
                    nc.tensor.matmul(
                        s_ps[:, 512 * q:512 * (q + 1)],
                        lhsT=eye16[:, 16 * k:16 * (k + 1)],
                        rhs=tt[:, par * HALF + 512 * q:par * HALF + 512 * (q + 1)],
                        start=(k == 0 and par == 0),
                        stop=(k == K - 1 and par == 1),
                    )

        nc.scalar.copy(ssb[:], s_ps[:])
        for t in range(NT):
            stp = ps_tp.tile([128, 16], f32, tag="tp")
            nc.tensor.transpose(stp[:], ssb[:, 128 * t:128 * (t + 1)], ident16)
            nc.scalar.copy(st[:, 16 * t:16 * (t + 1)], stp[:])

        # ---- argmax (first occurrence) + offset select ------------------------
        def grp(ap):  # (128, 256) -> (128, 16, 16)
            return ap.rearrange("p (t j) -> p t j", j=16)

        gmax = pers.tile([128, NT], f32, tag="gmax")
        ohall = pers.tile([128, 16 * NT], f32, tag="ohall")
        t1 = pers.tile([128, 16 * NT], f32, tag="t1")
        r1 = pers.tile([128, NT], f32, tag="r1")
        oh1 = pers.tile([128, 16 * NT], f32, tag="oh1")
        sel = pers.tile([128, 16 * NT], f32, tag="sel")
        oxv = pers.tile([128, NT], f32, tag="oxv")
        oyv = pers.tile([128, NT], f32, tag="oyv")

        nc.vector.tensor_reduce(gmax[:], grp(st[:]), axis=mybir.AxisListType.X,
                                op=mybir.AluOpType.max)
        gb = gmax[:].unsqueeze(2).broadcast_to((128, NT, 16))
        nc.vector.tensor_tensor(grp(ohall[:]), grp(st[:]), gb,
                                op=mybir.AluOpType.is_equal)
        nc.vector.tensor_mul(t1[:], ohall[:], revc[:])
        nc.vector.tensor_reduce(r1[:], grp(t1[:]), axis=mybir.AxisListType.X,
                                op=mybir.AluOpType.max)
        rb = r1[:].unsqueeze(2).broadcast_to((128, NT, 16))
        nc.vector.tensor_tensor(grp(oh1[:]), grp(t1[:]), rb,
                                op=mybir.AluOpType.is_equal)
        nc.vector.tensor_mul(sel[:], oh1[:], xt[:])
        nc.vector.tensor_reduce(oxv[:], grp(sel[:]), axis=mybir.AxisListType.X,
                                op=mybir.AluOpType.add)
        nc.vector.tensor_mul(sel[:], oh1[:], yt[:])
        nc.vector.tensor_reduce(oyv[:], grp(sel[:]), axis=mybir.AxisListType.X,
                                op=mybir.AluOpType.add)

        nc.sync.dma_start(ox_d.ap(), oxv[:])
        nc.sync.dma_start(oy_d.ap(), oyv[:])

    nc.compile()
    return nc


def make_consts():
    ident = np.eye(128, dtype=np.float32)
    eye16 = np.tile(np.eye(16, dtype=np.float32).reshape(1, 256), (128, 1))
    revc = np.tile(
        np.tile((16.0 - np.arange(16, dtype=np.float32)), NT).reshape(1, 16 * NT),
        (128, 1),
    )
    return ident, eye16, np.ascontiguousarray(revc)


def pack_pairs(a):
    # (256, n) -> (128, 2n): partition p holds channels 2p, 2p+1 interleaved
    cn, n = a.shape
    return np.ascontiguousarray(
        a.reshape(cn // 2, 2, n).transpose(0, 2, 1).reshape(cn // 2, 2 * n))


def make_in_maps(input_features, ref_features, aggregated_x, aggregated_y):
    ident, eye16, revc = make_consts()
    in_maps = []
    for core in range(NCORES):
        b, h = core // 2, core % 2
        sl = slice(h * HALF, (h + 1) * HALF)
        in_maps.append({
            "ref": pack_pairs(ref_features[b].reshape(CN, HW)),
            "inp": pack_pairs(input_features[b].reshape(CN, HW)[:, sl]),
            "xh": np.ascontiguousarray(aggregated_x[b].reshape(K, HW)[:, sl]),
            "yh": np.ascontiguousarray(aggregated_y[b].reshape(K, HW)[:, sl]),
            "ident": ident,
            "eye16": eye16,
            "revc": revc,
        })
    return in_maps


def assemble_outputs(results):
    offset_x = np.empty((B, 1, H, W), dtype=np.float32)
    offset_y = np.empty((B, 1, H, W), dtype=np.float32)
    for core in range(NCORES):
        b, h = core // 2, core % 2
        sl = slice(h * HALF, (h + 1) * HALF)
        # ox[p, t] holds pixel t*128+p -> transpose to pixel order
        offset_x[b, 0].reshape(HW)[sl] = results[core]["ox"].T.reshape(HALF)
        offset_y[b, 0].reshape(HW)[sl] = results[core]["oy"].T.reshape(HALF)
    return offset_x, offset_y


_PROGRAM = None


def kernel(input_features, ref_features, aggregated_x, aggregated_y):
    global _PROGRAM
    if _PROGRAM is None:
        _PROGRAM = build_program()
    nc = _PROGRAM
    in_maps = make_in_maps(input_features, ref_features, aggregated_x, aggregated_y)
    res = bass_utils.run_bass_kernel_spmd(nc, in_maps, core_ids=list(range(NCORES)))
    return assemble_outputs(res.results)
